# revision 18
# baseline (speedup 1.0000x reference)
"""RBF-kernel SVM decision function on 8 TRN2 NeuronCores.

out[i] = sum_j alphas[j] * exp(-GAMMA * ||x[i] - supports[j]||^2)

Strategy (data-parallel over x rows, supports/alphas replicated):
  exponent e_ij = -g|x_i|^2 + (2g x_i . s_j) + (ln|a_j| - g|s_j|^2)
    - 2g x_i.s_j  : bf16 matmul, x-side scaled by 1/32 (exact pow2), s-side raw
    - j-term      : folded into the matmul as 2 extra contraction rows (hi/lo
                    bf16 split for ~fp24 accuracy), x-side rows = 1.0
    - i-term      : fp32 per-partition bias of the ACTIVATE(Exp)
  out_i = sum_{j: a_j>0} exp(e_ij) - sum_{j: a_j<0} exp(e_ij)
    - supports host-sorted so positive-alpha group comes first

Two reduction schemes:
  dve_accum (default): ScalarE writes exp() to an fp16 SBUF staging buffer
    (64 plain ACTIVATEs, ScalarE floor ~128us/core); the otherwise-idle DVE
    reduces each sign group with tensor_scalar(accum_out=...) which runs at
    2-4x on 16-bit SBUF data. Sign boundary is rounded up to a multiple of 4
    (fast-mode alignment) by summing <=3 negative-alpha supports into the
    positive group and subtracting twice their contribution afterwards.
  act_accum (fallback, BASS_ACT_ACCUM=1): ACTIVATE(accum_out=...) reduces in
    the same pass; costs an ACTIVATION_READ_ACCUMULATOR per window plus a
    split ACTIVATE in the sign-boundary window.
"""

import os
import sys

for p in ("/opt/trn_rl_repo",):
    if p not in sys.path:
        sys.path.insert(0, p)

import numpy as np
import ml_dtypes

import concourse.bass as bass
import concourse.tile as tile
from concourse import bacc, mybir
from concourse.bass_utils import run_bass_kernel_spmd

N_CORES = 8
N = 16384
M = 8192
F = 64
GAMMA = 1.0 / F
N_LOC = N // N_CORES        # 2048 queries per core
N_TILES = N_LOC // 128      # 16 i-tiles of 128 queries
K_AUG = F + 2               # 66 contraction rows
W = 2048                    # j-window: 4 PSUM banks
NW = M // W                 # 4 windows per j sweep
MM_N = 512                  # matmul moving free dim (1 PSUM bank)
M_PAD = M + 256             # fp16 staging width (zero tail pad, mult of 4)

BF16 = mybir.dt.bfloat16
FP16 = mybir.dt.float16
F32 = mybir.dt.float32
bf16 = ml_dtypes.bfloat16

_compiled_cache = {}


def _build_common(nc, tc, cpool):
    """Input DRAM tensors, table-load warmer, and input DMAs (sync-ordered so
    the first window's operands land first)."""
    xaugT_d = nc.dram_tensor("xaugT", [K_AUG, N_LOC], BF16, kind="ExternalInput")
    saug_d = nc.dram_tensor("saug", [K_AUG, M], BF16, kind="ExternalInput")
    cbias_d = nc.dram_tensor("cbias", [128, N_TILES], F32, kind="ExternalInput")
    out_d = nc.dram_tensor("out", [128, N_TILES], F32, kind="ExternalOutput")

    # Dummy exp() on a zeroed tile: first in the ACT engine's program, so the
    # ~2.7us exp table load overlaps the input DMAs instead of stalling the
    # first real ACTIVATE.
    warm_act = cpool.tile([128, 1], F32)
    nc.gpsimd.memset(warm_act[:], 0.0)
    nc.scalar.activation(warm_act[:], warm_act[:], mybir.ActivationFunctionType.Exp)

    saug_sb = cpool.tile([K_AUG, M], BF16)
    nc.sync.dma_start(saug_sb[:, 0:W], saug_d.ap()[:, 0:W])
    xaugT_sb = cpool.tile([K_AUG, N_LOC], BF16)
    nc.sync.dma_start(xaugT_sb[:, 0:128], xaugT_d.ap()[:, 0:128])
    cbias_sb = cpool.tile([128, N_TILES], F32)
    nc.sync.dma_start(cbias_sb[:], cbias_d.ap()[:])
    for w in range(1, NW):
        nc.sync.dma_start(
            saug_sb[:, w * W : (w + 1) * W],
            saug_d.ap()[:, w * W : (w + 1) * W],
        )
    nc.sync.dma_start(xaugT_sb[:, 128:], xaugT_d.ap()[:, 128:])
    return xaugT_sb, saug_sb, cbias_sb, out_d


def _mm_windows(nc, t, ps_tile, w, xaugT_sb, saug_sb):
    for c in range(W // MM_N):
        nc.tensor.matmul(
            ps_tile[:, c * MM_N : (c + 1) * MM_N],
            xaugT_sb[:, t * 128 : (t + 1) * 128],
            saug_sb[:, w * W + c * MM_N : w * W + (c + 1) * MM_N],
            start=True,
            stop=True,
        )


def _build_dve_accum(b):
    """Hybrid reduction: one window per i-tile uses ACTIVATE(accum_out=...)
    (in-place on PSUM, one accumulator read); the other three are written as
    fp16 to SBUF staging and reduced by the otherwise-idle DVE with 1x
    TENSOR_SCALAR_CACHE_REDUCE ops. The sign-boundary window goes to the DVE,
    where an arbitrary split point costs only one extra small op."""
    nc = bacc.Bacc(
        "TRN2",
        target_bir_lowering=False,
        debug=False,
        enable_asserts=False,
        num_devices=N_CORES,
    )
    w_mix = b // W  # window containing the P/N boundary (b % W may be 0)
    act_w = 0 if w_mix != 0 else 1  # the one ACT-accum window, never mixed
    dve_ws = [w for w in range(NW) if w != act_w]

    def pieces_of(w):
        lo, hi = w * W, (w + 1) * W
        if b <= lo:
            return [(lo, hi, False)]
        if b >= hi:
            return [(lo, hi, True)]
        return [(lo, b, True), (b, hi, False)]

    n_pos = sum(1 for w in range(NW) for p in pieces_of(w) if p[2])
    n_neg = sum(1 for w in range(NW) for p in pieces_of(w) if not p[2])

    with tile.TileContext(nc) as tc:
        with (
            tc.tile_pool(name="const", bufs=1) as cpool,
            tc.tile_pool(name="acc", bufs=3) as apool,
            tc.tile_pool(name="stg", bufs=2) as spool,
            tc.tile_pool(name="psum", bufs=2, space="PSUM") as ppool,
        ):
            xaugT_sb, saug_sb, cbias_sb, out_d = _build_common(nc, tc, cpool)
            outT_sb = cpool.tile([128, N_TILES], F32)
            dvout = cpool.tile([128, M], FP16)

            for t in range(N_TILES):
                accP = apool.tile([128, max(n_pos, 1)], F32, tag="accP")
                accN = apool.tile([128, max(n_neg, 1)], F32, tag="accN")
                iP = iN = 0

                def acc_col(pos):
                    nonlocal iP, iN
                    if pos:
                        col = accP[:, iP : iP + 1]
                        iP += 1
                    else:
                        col = accN[:, iN : iN + 1]
                        iN += 1
                    return col

                # Last i-tile: the trailing windows go through ACT-accum so no
                # DVE reduce chain trails the final ACTIVATE; earlier windows'
                # DVE reduces overlap the remaining ACTIVATEs as usual.
                last = t == N_TILES - 1
                act_ws = {act_w, NW - 1} if last else {act_w}
                stg = spool.tile([128, M], FP16, tag="stg")
                for w in range(NW):
                    ps_tile = ppool.tile([128, W], F32, tag="E")
                    _mm_windows(nc, t, ps_tile, w, xaugT_sb, saug_sb)
                    if w in act_ws:
                        for lo, hi, pos in pieces_of(w):
                            nc.scalar.activation(
                                ps_tile[:, lo - w * W : hi - w * W],
                                ps_tile[:, lo - w * W : hi - w * W],
                                mybir.ActivationFunctionType.Exp,
                                bias=cbias_sb[:, t : t + 1],
                                accum_out=acc_col(pos),
                            )
                    else:
                        nc.scalar.activation(
                            stg[:, w * W : (w + 1) * W],
                            ps_tile[:],
                            mybir.ActivationFunctionType.Exp,
                            bias=cbias_sb[:, t : t + 1],
                        )
                for w in dve_ws:
                    if w in act_ws:
                        continue
                    for lo, hi, pos in pieces_of(w):
                        nc.vector.tensor_scalar(
                            dvout[:, lo:hi],
                            stg[:, lo:hi],
                            1.0,
                            0.0,
                            mybir.AluOpType.mult,
                            mybir.AluOpType.add,
                            accum_out=acc_col(pos),
                        )
                sumP = apool.tile([128, 1], F32, tag="sumP")
                nc.vector.reduce_sum(sumP[:], accP[:, :iP], axis=mybir.AxisListType.X)
                sumN = apool.tile([128, 1], F32, tag="sumN")
                nc.vector.reduce_sum(sumN[:], accN[:, :iN], axis=mybir.AxisListType.X)
                nc.vector.tensor_sub(outT_sb[:, t : t + 1], sumP[:], sumN[:])

            nc.sync.dma_start(out_d.ap()[:], outT_sb[:])

    nc.compile()
    return nc


def _build_act_accum(b):
    nc = bacc.Bacc(
        "TRN2",
        target_bir_lowering=False,
        debug=False,
        enable_asserts=False,
        num_devices=N_CORES,
    )
    n_pos = sum(1 for w in range(NW) if w * W < b)
    n_neg = sum(1 for w in range(NW) if (w + 1) * W > b)

    with tile.TileContext(nc) as tc:
        with (
            tc.tile_pool(name="const", bufs=1) as cpool,
            tc.tile_pool(name="acc", bufs=3) as apool,
            tc.tile_pool(name="psum", bufs=2, space="PSUM") as ppool,
        ):
            xaugT_sb, saug_sb, cbias_sb, out_d = _build_common(nc, tc, cpool)
            outT_sb = cpool.tile([128, N_TILES], F32)

            for t in range(N_TILES):
                accP = apool.tile([128, max(n_pos, 1)], F32, tag="accP")
                accN = apool.tile([128, max(n_neg, 1)], F32, tag="accN")
                iP = iN = 0
                for w in range(NW):
                    ps_tile = ppool.tile([128, W], F32, tag="E")
                    _mm_windows(nc, t, ps_tile, w, xaugT_sb, saug_sb)
                    lo, hi = w * W, (w + 1) * W
                    if b <= lo:
                        pieces = [(lo, hi, False)]
                    elif b >= hi:
                        pieces = [(lo, hi, True)]
                    else:
                        pieces = [(lo, b, True), (b, hi, False)]
                    for plo, phi, pos in pieces:
                        if pos:
                            acc_col = accP[:, iP : iP + 1]
                            iP += 1
                        else:
                            acc_col = accN[:, iN : iN + 1]
                            iN += 1
                        nc.scalar.activation(
                            ps_tile[:, plo - lo : phi - lo],
                            ps_tile[:, plo - lo : phi - lo],
                            mybir.ActivationFunctionType.Exp,
                            bias=cbias_sb[:, t : t + 1],
                            accum_out=acc_col,
                        )
                sumP = apool.tile([128, 1], F32, tag="sumP")
                nc.vector.reduce_sum(sumP[:], accP[:, :iP], axis=mybir.AxisListType.X)
                sumN = apool.tile([128, 1], F32, tag="sumN")
                nc.vector.reduce_sum(sumN[:], accN[:, :iN], axis=mybir.AxisListType.X)
                nc.vector.tensor_sub(outT_sb[:, t : t + 1], sumP[:], sumN[:])

            nc.sync.dma_start(out_d.ap()[:], outT_sb[:])

    nc.compile()
    return nc


def _build(b):
    if os.environ.get("BASS_ACT_ACCUM"):
        return _build_act_accum(b)
    return _build_dve_accum(b)


def _prepare(x, supports, alphas):
    x = np.asarray(x, dtype=np.float32)
    supports = np.asarray(supports, dtype=np.float32)
    alphas = np.asarray(alphas, dtype=np.float32)

    a64 = alphas.astype(np.float64)
    s64 = supports.astype(np.float64)
    jterm = -GAMMA * (s64 * s64).sum(axis=1) + np.log(
        np.maximum(np.abs(a64), 1e-300)
    )

    pos = a64 > 0
    perm = np.concatenate([np.nonzero(pos)[0], np.nonzero(~pos)[0]])
    b = int(pos.sum())

    jt = jterm[perm]
    hi = jt.astype(bf16)
    lo = (jt - hi.astype(np.float64)).astype(bf16)

    saug = np.empty((K_AUG, M), dtype=bf16)
    saug[:F] = supports[perm].T.astype(bf16)
    saug[F] = hi
    saug[F + 1] = lo

    xaugT = np.ones((K_AUG, N), dtype=bf16)
    xaugT[:F] = (x.T / 32.0).astype(bf16)

    cbias = (-GAMMA * (x.astype(np.float64) ** 2).sum(axis=1)).astype(np.float32)

    in_maps = []
    for c in range(N_CORES):
        sl = slice(c * N_LOC, (c + 1) * N_LOC)
        in_maps.append(
            {
                "xaugT": np.ascontiguousarray(xaugT[:, sl]),
                "saug": saug,
                "cbias": np.ascontiguousarray(
                    cbias[sl].reshape(N_TILES, 128).T
                ),
            }
        )
    return b, in_maps


def _run(x, supports, alphas, trace=False, **run_kwargs):
    b, in_maps = _prepare(x, supports, alphas)
    key = (b, bool(os.environ.get("BASS_ACT_ACCUM")))
    if key not in _compiled_cache:
        _compiled_cache[key] = _build(b)
    nc = _compiled_cache[key]
    res = run_bass_kernel_spmd(
        nc, in_maps, core_ids=list(range(N_CORES)), trace=trace, **run_kwargs
    )
    outs = [r["out"].T.reshape(-1) for r in res.results]
    return np.concatenate(outs).astype(np.float32), res


def kernel(x, supports, alphas):
    out, _ = _run(x, supports, alphas, trace=False)
    return out


# revision 19
# speedup vs baseline: 1.0054x; 1.0054x over previous
"""RBF-kernel SVM decision function on 8 TRN2 NeuronCores.

out[i] = sum_j alphas[j] * exp(-GAMMA * ||x[i] - supports[j]||^2)

Strategy (data-parallel over x rows, supports/alphas replicated):
  exponent e_ij = -g|x_i|^2 + (2g x_i . s_j) + (ln|a_j| - g|s_j|^2)
    - 2g x_i.s_j  : bf16 matmul, x-side scaled by 1/32 (exact pow2), s-side raw
    - j-term      : folded into the matmul as 2 extra contraction rows (hi/lo
                    bf16 split for ~fp24 accuracy), x-side rows = 1.0
    - i-term      : fp32 per-partition bias of the ACTIVATE(Exp)
  out_i = sum_{j: a_j>0} exp(e_ij) - sum_{j: a_j<0} exp(e_ij)
    - supports host-sorted so positive-alpha group comes first

Two reduction schemes:
  dve_accum (default): ScalarE writes exp() to an fp16 SBUF staging buffer
    (64 plain ACTIVATEs, ScalarE floor ~128us/core); the otherwise-idle DVE
    reduces each sign group with tensor_scalar(accum_out=...) which runs at
    2-4x on 16-bit SBUF data. Sign boundary is rounded up to a multiple of 4
    (fast-mode alignment) by summing <=3 negative-alpha supports into the
    positive group and subtracting twice their contribution afterwards.
  act_accum (fallback, BASS_ACT_ACCUM=1): ACTIVATE(accum_out=...) reduces in
    the same pass; costs an ACTIVATION_READ_ACCUMULATOR per window plus a
    split ACTIVATE in the sign-boundary window.
"""

import os
import sys

for p in ("/opt/trn_rl_repo",):
    if p not in sys.path:
        sys.path.insert(0, p)

import numpy as np
import ml_dtypes

import concourse.bass as bass
import concourse.tile as tile
from concourse import bacc, mybir
from concourse.bass_utils import run_bass_kernel_spmd

N_CORES = 8
N = 16384
M = 8192
F = 64
GAMMA = 1.0 / F
N_LOC = N // N_CORES        # 2048 queries per core
N_TILES = N_LOC // 128      # 16 i-tiles of 128 queries
K_AUG = F + 2               # 66 contraction rows
W = 2048                    # j-window: 4 PSUM banks
NW = M // W                 # 4 windows per j sweep
MM_N = 512                  # matmul moving free dim (1 PSUM bank)
M_PAD = M + 256             # fp16 staging width (zero tail pad, mult of 4)

BF16 = mybir.dt.bfloat16
FP16 = mybir.dt.float16
F32 = mybir.dt.float32
bf16 = ml_dtypes.bfloat16

_compiled_cache = {}


def _build_common(nc, tc, cpool):
    """Input DRAM tensors, table-load warmer, and input DMAs (sync-ordered so
    the first window's operands land first)."""
    xaugT_d = nc.dram_tensor("xaugT", [K_AUG, N_LOC], BF16, kind="ExternalInput")
    saug_d = nc.dram_tensor("saug", [K_AUG, M], BF16, kind="ExternalInput")
    cbias_d = nc.dram_tensor("cbias", [128, N_TILES], F32, kind="ExternalInput")
    out_d = nc.dram_tensor("out", [128, N_TILES], F32, kind="ExternalOutput")

    # Dummy exp() on a zeroed tile: first in the ACT engine's program, so the
    # ~2.7us exp table load overlaps the input DMAs instead of stalling the
    # first real ACTIVATE.
    warm_act = cpool.tile([128, 1], F32)
    nc.gpsimd.memset(warm_act[:], 0.0)
    nc.scalar.activation(warm_act[:], warm_act[:], mybir.ActivationFunctionType.Exp)

    saug_sb = cpool.tile([K_AUG, M], BF16)
    nc.sync.dma_start(saug_sb[:, 0:W], saug_d.ap()[:, 0:W])
    xaugT_sb = cpool.tile([K_AUG, N_LOC], BF16)
    nc.sync.dma_start(xaugT_sb[:, 0:128], xaugT_d.ap()[:, 0:128])
    cbias_sb = cpool.tile([128, N_TILES], F32)
    nc.sync.dma_start(cbias_sb[:], cbias_d.ap()[:])
    for w in range(1, NW):
        nc.sync.dma_start(
            saug_sb[:, w * W : (w + 1) * W],
            saug_d.ap()[:, w * W : (w + 1) * W],
        )
    nc.sync.dma_start(xaugT_sb[:, 128:], xaugT_d.ap()[:, 128:])
    return xaugT_sb, saug_sb, cbias_sb, out_d


def _mm_windows(nc, t, ps_tile, w, xaugT_sb, saug_sb):
    for c in range(W // MM_N):
        nc.tensor.matmul(
            ps_tile[:, c * MM_N : (c + 1) * MM_N],
            xaugT_sb[:, t * 128 : (t + 1) * 128],
            saug_sb[:, w * W + c * MM_N : w * W + (c + 1) * MM_N],
            start=True,
            stop=True,
        )


def _build_dve_accum(b):
    """Hybrid reduction: one window per i-tile uses ACTIVATE(accum_out=...)
    (in-place on PSUM, one accumulator read); the other three are written as
    fp16 to SBUF staging and reduced by the otherwise-idle DVE with 1x
    TENSOR_SCALAR_CACHE_REDUCE ops. The sign-boundary window goes to the DVE,
    where an arbitrary split point costs only one extra small op."""
    nc = bacc.Bacc(
        "TRN2",
        target_bir_lowering=False,
        debug=False,
        enable_asserts=False,
        num_devices=N_CORES,
    )
    w_mix = b // W  # window containing the P/N boundary (b % W may be 0)
    act_w = 0 if w_mix != 0 else 1  # the one ACT-accum window, never mixed
    dve_ws = [w for w in range(NW) if w != act_w]

    def pieces_of(w):
        lo, hi = w * W, (w + 1) * W
        if b <= lo:
            return [(lo, hi, False)]
        if b >= hi:
            return [(lo, hi, True)]
        return [(lo, b, True), (b, hi, False)]

    n_pos = sum(1 for w in range(NW) for p in pieces_of(w) if p[2])
    n_neg = sum(1 for w in range(NW) for p in pieces_of(w) if not p[2])

    with tile.TileContext(nc) as tc:
        with (
            tc.tile_pool(name="const", bufs=1) as cpool,
            tc.tile_pool(name="acc", bufs=3) as apool,
            tc.tile_pool(name="stg", bufs=2) as spool,
            tc.tile_pool(name="psum", bufs=2, space="PSUM") as ppool,
        ):
            xaugT_sb, saug_sb, cbias_sb, out_d = _build_common(nc, tc, cpool)
            outT_sb = cpool.tile([128, N_TILES], F32)
            dvout = cpool.tile([128, M], FP16)

            for t in range(N_TILES):
                accP = apool.tile([128, max(n_pos, 1)], F32, tag="accP")
                accN = apool.tile([128, max(n_neg, 1)], F32, tag="accN")
                iP = iN = 0

                def acc_col(pos):
                    nonlocal iP, iN
                    if pos:
                        col = accP[:, iP : iP + 1]
                        iP += 1
                    else:
                        col = accN[:, iN : iN + 1]
                        iN += 1
                    return col

                # Last i-tile: ACT-accum everything (split at the sign
                # boundary) so no DVE reduce chain trails the final ACTIVATE.
                last = t == N_TILES - 1
                stg = spool.tile([128, M], FP16, tag="stg")
                for w in range(NW):
                    ps_tile = ppool.tile([128, W], F32, tag="E")
                    _mm_windows(nc, t, ps_tile, w, xaugT_sb, saug_sb)
                    if last or w == act_w:
                        for lo, hi, pos in pieces_of(w) if last else [
                            (w * W, (w + 1) * W, w * W < b)
                        ]:
                            nc.scalar.activation(
                                ps_tile[:, lo - w * W : hi - w * W],
                                ps_tile[:, lo - w * W : hi - w * W],
                                mybir.ActivationFunctionType.Exp,
                                bias=cbias_sb[:, t : t + 1],
                                accum_out=acc_col(pos),
                            )
                    else:
                        nc.scalar.activation(
                            stg[:, w * W : (w + 1) * W],
                            ps_tile[:],
                            mybir.ActivationFunctionType.Exp,
                            bias=cbias_sb[:, t : t + 1],
                        )
                if not last:
                    for w in dve_ws:
                        for lo, hi, pos in pieces_of(w):
                            nc.vector.tensor_scalar(
                                dvout[:, lo:hi],
                                stg[:, lo:hi],
                                1.0,
                                0.0,
                                mybir.AluOpType.mult,
                                mybir.AluOpType.add,
                                accum_out=acc_col(pos),
                            )
                sumP = apool.tile([128, 1], F32, tag="sumP")
                nc.vector.reduce_sum(sumP[:], accP[:, :iP], axis=mybir.AxisListType.X)
                sumN = apool.tile([128, 1], F32, tag="sumN")
                nc.vector.reduce_sum(sumN[:], accN[:, :iN], axis=mybir.AxisListType.X)
                nc.vector.tensor_sub(outT_sb[:, t : t + 1], sumP[:], sumN[:])

            nc.sync.dma_start(out_d.ap()[:], outT_sb[:])

    nc.compile()
    return nc


def _build_act_accum(b):
    nc = bacc.Bacc(
        "TRN2",
        target_bir_lowering=False,
        debug=False,
        enable_asserts=False,
        num_devices=N_CORES,
    )
    n_pos = sum(1 for w in range(NW) if w * W < b)
    n_neg = sum(1 for w in range(NW) if (w + 1) * W > b)

    with tile.TileContext(nc) as tc:
        with (
            tc.tile_pool(name="const", bufs=1) as cpool,
            tc.tile_pool(name="acc", bufs=3) as apool,
            tc.tile_pool(name="psum", bufs=2, space="PSUM") as ppool,
        ):
            xaugT_sb, saug_sb, cbias_sb, out_d = _build_common(nc, tc, cpool)
            outT_sb = cpool.tile([128, N_TILES], F32)

            for t in range(N_TILES):
                accP = apool.tile([128, max(n_pos, 1)], F32, tag="accP")
                accN = apool.tile([128, max(n_neg, 1)], F32, tag="accN")
                iP = iN = 0
                for w in range(NW):
                    ps_tile = ppool.tile([128, W], F32, tag="E")
                    _mm_windows(nc, t, ps_tile, w, xaugT_sb, saug_sb)
                    lo, hi = w * W, (w + 1) * W
                    if b <= lo:
                        pieces = [(lo, hi, False)]
                    elif b >= hi:
                        pieces = [(lo, hi, True)]
                    else:
                        pieces = [(lo, b, True), (b, hi, False)]
                    for plo, phi, pos in pieces:
                        if pos:
                            acc_col = accP[:, iP : iP + 1]
                            iP += 1
                        else:
                            acc_col = accN[:, iN : iN + 1]
                            iN += 1
                        nc.scalar.activation(
                            ps_tile[:, plo - lo : phi - lo],
                            ps_tile[:, plo - lo : phi - lo],
                            mybir.ActivationFunctionType.Exp,
                            bias=cbias_sb[:, t : t + 1],
                            accum_out=acc_col,
                        )
                sumP = apool.tile([128, 1], F32, tag="sumP")
                nc.vector.reduce_sum(sumP[:], accP[:, :iP], axis=mybir.AxisListType.X)
                sumN = apool.tile([128, 1], F32, tag="sumN")
                nc.vector.reduce_sum(sumN[:], accN[:, :iN], axis=mybir.AxisListType.X)
                nc.vector.tensor_sub(outT_sb[:, t : t + 1], sumP[:], sumN[:])

            nc.sync.dma_start(out_d.ap()[:], outT_sb[:])

    nc.compile()
    return nc


def _build(b):
    if os.environ.get("BASS_ACT_ACCUM"):
        return _build_act_accum(b)
    return _build_dve_accum(b)


def _prepare(x, supports, alphas):
    x = np.asarray(x, dtype=np.float32)
    supports = np.asarray(supports, dtype=np.float32)
    alphas = np.asarray(alphas, dtype=np.float32)

    a64 = alphas.astype(np.float64)
    s64 = supports.astype(np.float64)
    jterm = -GAMMA * (s64 * s64).sum(axis=1) + np.log(
        np.maximum(np.abs(a64), 1e-300)
    )

    pos = a64 > 0
    perm = np.concatenate([np.nonzero(pos)[0], np.nonzero(~pos)[0]])
    b = int(pos.sum())

    jt = jterm[perm]
    hi = jt.astype(bf16)
    lo = (jt - hi.astype(np.float64)).astype(bf16)

    saug = np.empty((K_AUG, M), dtype=bf16)
    saug[:F] = supports[perm].T.astype(bf16)
    saug[F] = hi
    saug[F + 1] = lo

    xaugT = np.ones((K_AUG, N), dtype=bf16)
    xaugT[:F] = (x.T / 32.0).astype(bf16)

    cbias = (-GAMMA * (x.astype(np.float64) ** 2).sum(axis=1)).astype(np.float32)

    in_maps = []
    for c in range(N_CORES):
        sl = slice(c * N_LOC, (c + 1) * N_LOC)
        in_maps.append(
            {
                "xaugT": np.ascontiguousarray(xaugT[:, sl]),
                "saug": saug,
                "cbias": np.ascontiguousarray(
                    cbias[sl].reshape(N_TILES, 128).T
                ),
            }
        )
    return b, in_maps


def _run(x, supports, alphas, trace=False, **run_kwargs):
    b, in_maps = _prepare(x, supports, alphas)
    key = (b, bool(os.environ.get("BASS_ACT_ACCUM")))
    if key not in _compiled_cache:
        _compiled_cache[key] = _build(b)
    nc = _compiled_cache[key]
    res = run_bass_kernel_spmd(
        nc, in_maps, core_ids=list(range(N_CORES)), trace=trace, **run_kwargs
    )
    outs = [r["out"].T.reshape(-1) for r in res.results]
    return np.concatenate(outs).astype(np.float32), res


def kernel(x, supports, alphas):
    out, _ = _run(x, supports, alphas, trace=False)
    return out


# revision 20
# speedup vs baseline: 1.0098x; 1.0044x over previous
"""RBF-kernel SVM decision function on 8 TRN2 NeuronCores.

out[i] = sum_j alphas[j] * exp(-GAMMA * ||x[i] - supports[j]||^2)

Strategy (data-parallel over x rows, supports/alphas replicated):
  exponent e_ij = -g|x_i|^2 + (2g x_i . s_j) + (ln|a_j| - g|s_j|^2)
    - 2g x_i.s_j  : bf16 matmul, x-side scaled by 1/32 (exact pow2), s-side raw
    - j-term      : folded into the matmul as 2 extra contraction rows (hi/lo
                    bf16 split for ~fp24 accuracy), x-side rows = 1.0
    - i-term      : fp32 per-partition bias of the ACTIVATE(Exp)
  out_i = sum_{j: a_j>0} exp(e_ij) - sum_{j: a_j<0} exp(e_ij)
    - supports host-sorted so positive-alpha group comes first

Two reduction schemes:
  dve_accum (default): ScalarE writes exp() to an fp16 SBUF staging buffer
    (64 plain ACTIVATEs, ScalarE floor ~128us/core); the otherwise-idle DVE
    reduces each sign group with tensor_scalar(accum_out=...) which runs at
    2-4x on 16-bit SBUF data. Sign boundary is rounded up to a multiple of 4
    (fast-mode alignment) by summing <=3 negative-alpha supports into the
    positive group and subtracting twice their contribution afterwards.
  act_accum (fallback, BASS_ACT_ACCUM=1): ACTIVATE(accum_out=...) reduces in
    the same pass; costs an ACTIVATION_READ_ACCUMULATOR per window plus a
    split ACTIVATE in the sign-boundary window.
"""

import os
import sys

for p in ("/opt/trn_rl_repo",):
    if p not in sys.path:
        sys.path.insert(0, p)

import numpy as np
import ml_dtypes

import concourse.bass as bass
import concourse.tile as tile
from concourse import bacc, mybir
from concourse.bass_utils import run_bass_kernel_spmd

N_CORES = 8
N = 16384
M = 8192
F = 64
GAMMA = 1.0 / F
N_LOC = N // N_CORES        # 2048 queries per core
N_TILES = N_LOC // 128      # 16 i-tiles of 128 queries
K_AUG = F + 2               # 66 contraction rows
W = 2048                    # j-window: 4 PSUM banks
NW = M // W                 # 4 windows per j sweep
MM_N = 512                  # matmul moving free dim (1 PSUM bank)
M_PAD = M + 256             # fp16 staging width (zero tail pad, mult of 4)

BF16 = mybir.dt.bfloat16
FP16 = mybir.dt.float16
F32 = mybir.dt.float32
bf16 = ml_dtypes.bfloat16

_compiled_cache = {}


def _build_common(nc, tc, cpool):
    """Input DRAM tensors, table-load warmer, and input DMAs (sync-ordered so
    the first window's operands land first)."""
    xaugT_d = nc.dram_tensor("xaugT", [K_AUG, N_LOC], BF16, kind="ExternalInput")
    saug_d = nc.dram_tensor("saug", [K_AUG, M], BF16, kind="ExternalInput")
    cbias_d = nc.dram_tensor("cbias", [128, N_TILES], F32, kind="ExternalInput")
    out_d = nc.dram_tensor("out", [128, N_TILES], F32, kind="ExternalOutput")

    # Dummy exp() on a zeroed tile: first in the ACT engine's program, so the
    # ~2.7us exp table load overlaps the input DMAs instead of stalling the
    # first real ACTIVATE.
    warm_act = cpool.tile([128, 1], F32)
    nc.gpsimd.memset(warm_act[:], 0.0)
    nc.scalar.activation(warm_act[:], warm_act[:], mybir.ActivationFunctionType.Exp)

    saug_sb = cpool.tile([K_AUG, M], BF16)
    nc.sync.dma_start(saug_sb[:, 0:W], saug_d.ap()[:, 0:W])
    xaugT_sb = cpool.tile([K_AUG, N_LOC], BF16)
    nc.sync.dma_start(xaugT_sb[:, 0:128], xaugT_d.ap()[:, 0:128])
    cbias_sb = cpool.tile([128, N_TILES], F32)
    nc.sync.dma_start(cbias_sb[:], cbias_d.ap()[:])
    for w in range(1, NW):
        nc.sync.dma_start(
            saug_sb[:, w * W : (w + 1) * W],
            saug_d.ap()[:, w * W : (w + 1) * W],
        )
    nc.sync.dma_start(xaugT_sb[:, 128:], xaugT_d.ap()[:, 128:])
    return xaugT_sb, saug_sb, cbias_sb, out_d


def _mm_windows(nc, t, ps_tile, w, xaugT_sb, saug_sb):
    for c in range(W // MM_N):
        nc.tensor.matmul(
            ps_tile[:, c * MM_N : (c + 1) * MM_N],
            xaugT_sb[:, t * 128 : (t + 1) * 128],
            saug_sb[:, w * W + c * MM_N : w * W + (c + 1) * MM_N],
            start=True,
            stop=True,
        )


def _build_dve_accum(b):
    """Hybrid reduction: one window per i-tile uses ACTIVATE(accum_out=...)
    (in-place on PSUM, one accumulator read); the other three are written as
    fp16 to SBUF staging and reduced by the otherwise-idle DVE with 1x
    TENSOR_SCALAR_CACHE_REDUCE ops. The sign-boundary window goes to the DVE,
    where an arbitrary split point costs only one extra small op."""
    nc = bacc.Bacc(
        "TRN2",
        target_bir_lowering=False,
        debug=False,
        enable_asserts=False,
        num_devices=N_CORES,
    )
    w_mix = b // W  # window containing the P/N boundary (b % W may be 0)
    act_w = 0 if w_mix != 0 else 1  # the one ACT-accum window, never mixed
    dve_ws = [w for w in range(NW) if w != act_w]

    def pieces_of(w):
        lo, hi = w * W, (w + 1) * W
        if b <= lo:
            return [(lo, hi, False)]
        if b >= hi:
            return [(lo, hi, True)]
        return [(lo, b, True), (b, hi, False)]

    n_pos = sum(1 for w in range(NW) for p in pieces_of(w) if p[2])
    n_neg = sum(1 for w in range(NW) for p in pieces_of(w) if not p[2])

    with tile.TileContext(nc) as tc:
        with (
            tc.tile_pool(name="const", bufs=1) as cpool,
            tc.tile_pool(name="acc", bufs=3) as apool,
            tc.tile_pool(name="stg", bufs=3) as spool,
            tc.tile_pool(name="psum", bufs=2, space="PSUM") as ppool,
        ):
            xaugT_sb, saug_sb, cbias_sb, out_d = _build_common(nc, tc, cpool)
            outT_sb = cpool.tile([128, N_TILES], F32)
            dvout = cpool.tile([128, M], FP16)

            for t in range(N_TILES):
                accP = apool.tile([128, max(n_pos, 1)], F32, tag="accP")
                accN = apool.tile([128, max(n_neg, 1)], F32, tag="accN")
                iP = iN = 0

                def acc_col(pos):
                    nonlocal iP, iN
                    if pos:
                        col = accP[:, iP : iP + 1]
                        iP += 1
                    else:
                        col = accN[:, iN : iN + 1]
                        iN += 1
                    return col

                # Last i-tile: ACT-accum everything (split at the sign
                # boundary) so no DVE reduce chain trails the final ACTIVATE.
                last = t == N_TILES - 1
                stg = spool.tile([128, M], FP16, tag="stg")
                for w in range(NW):
                    ps_tile = ppool.tile([128, W], F32, tag="E")
                    _mm_windows(nc, t, ps_tile, w, xaugT_sb, saug_sb)
                    if last or w == act_w:
                        for lo, hi, pos in pieces_of(w) if last else [
                            (w * W, (w + 1) * W, w * W < b)
                        ]:
                            nc.scalar.activation(
                                ps_tile[:, lo - w * W : hi - w * W],
                                ps_tile[:, lo - w * W : hi - w * W],
                                mybir.ActivationFunctionType.Exp,
                                bias=cbias_sb[:, t : t + 1],
                                accum_out=acc_col(pos),
                            )
                    else:
                        nc.scalar.activation(
                            stg[:, w * W : (w + 1) * W],
                            ps_tile[:],
                            mybir.ActivationFunctionType.Exp,
                            bias=cbias_sb[:, t : t + 1],
                        )
                if not last:
                    for w in dve_ws:
                        for lo, hi, pos in pieces_of(w):
                            nc.vector.tensor_scalar(
                                dvout[:, lo:hi],
                                stg[:, lo:hi],
                                1.0,
                                0.0,
                                mybir.AluOpType.mult,
                                mybir.AluOpType.add,
                                accum_out=acc_col(pos),
                            )
                sumP = apool.tile([128, 1], F32, tag="sumP")
                nc.vector.reduce_sum(sumP[:], accP[:, :iP], axis=mybir.AxisListType.X)
                sumN = apool.tile([128, 1], F32, tag="sumN")
                nc.vector.reduce_sum(sumN[:], accN[:, :iN], axis=mybir.AxisListType.X)
                nc.vector.tensor_sub(outT_sb[:, t : t + 1], sumP[:], sumN[:])

            nc.sync.dma_start(out_d.ap()[:], outT_sb[:])

    nc.compile()
    return nc


def _build_act_accum(b):
    nc = bacc.Bacc(
        "TRN2",
        target_bir_lowering=False,
        debug=False,
        enable_asserts=False,
        num_devices=N_CORES,
    )
    n_pos = sum(1 for w in range(NW) if w * W < b)
    n_neg = sum(1 for w in range(NW) if (w + 1) * W > b)

    with tile.TileContext(nc) as tc:
        with (
            tc.tile_pool(name="const", bufs=1) as cpool,
            tc.tile_pool(name="acc", bufs=3) as apool,
            tc.tile_pool(name="psum", bufs=2, space="PSUM") as ppool,
        ):
            xaugT_sb, saug_sb, cbias_sb, out_d = _build_common(nc, tc, cpool)
            outT_sb = cpool.tile([128, N_TILES], F32)

            for t in range(N_TILES):
                accP = apool.tile([128, max(n_pos, 1)], F32, tag="accP")
                accN = apool.tile([128, max(n_neg, 1)], F32, tag="accN")
                iP = iN = 0
                for w in range(NW):
                    ps_tile = ppool.tile([128, W], F32, tag="E")
                    _mm_windows(nc, t, ps_tile, w, xaugT_sb, saug_sb)
                    lo, hi = w * W, (w + 1) * W
                    if b <= lo:
                        pieces = [(lo, hi, False)]
                    elif b >= hi:
                        pieces = [(lo, hi, True)]
                    else:
                        pieces = [(lo, b, True), (b, hi, False)]
                    for plo, phi, pos in pieces:
                        if pos:
                            acc_col = accP[:, iP : iP + 1]
                            iP += 1
                        else:
                            acc_col = accN[:, iN : iN + 1]
                            iN += 1
                        nc.scalar.activation(
                            ps_tile[:, plo - lo : phi - lo],
                            ps_tile[:, plo - lo : phi - lo],
                            mybir.ActivationFunctionType.Exp,
                            bias=cbias_sb[:, t : t + 1],
                            accum_out=acc_col,
                        )
                sumP = apool.tile([128, 1], F32, tag="sumP")
                nc.vector.reduce_sum(sumP[:], accP[:, :iP], axis=mybir.AxisListType.X)
                sumN = apool.tile([128, 1], F32, tag="sumN")
                nc.vector.reduce_sum(sumN[:], accN[:, :iN], axis=mybir.AxisListType.X)
                nc.vector.tensor_sub(outT_sb[:, t : t + 1], sumP[:], sumN[:])

            nc.sync.dma_start(out_d.ap()[:], outT_sb[:])

    nc.compile()
    return nc


def _build(b):
    if os.environ.get("BASS_ACT_ACCUM"):
        return _build_act_accum(b)
    return _build_dve_accum(b)


def _prepare(x, supports, alphas):
    x = np.asarray(x, dtype=np.float32)
    supports = np.asarray(supports, dtype=np.float32)
    alphas = np.asarray(alphas, dtype=np.float32)

    a64 = alphas.astype(np.float64)
    s64 = supports.astype(np.float64)
    jterm = -GAMMA * (s64 * s64).sum(axis=1) + np.log(
        np.maximum(np.abs(a64), 1e-300)
    )

    pos = a64 > 0
    perm = np.concatenate([np.nonzero(pos)[0], np.nonzero(~pos)[0]])
    b = int(pos.sum())

    jt = jterm[perm]
    hi = jt.astype(bf16)
    lo = (jt - hi.astype(np.float64)).astype(bf16)

    saug = np.empty((K_AUG, M), dtype=bf16)
    saug[:F] = supports[perm].T.astype(bf16)
    saug[F] = hi
    saug[F + 1] = lo

    xaugT = np.ones((K_AUG, N), dtype=bf16)
    xaugT[:F] = (x.T / 32.0).astype(bf16)

    cbias = (-GAMMA * (x.astype(np.float64) ** 2).sum(axis=1)).astype(np.float32)

    in_maps = []
    for c in range(N_CORES):
        sl = slice(c * N_LOC, (c + 1) * N_LOC)
        in_maps.append(
            {
                "xaugT": np.ascontiguousarray(xaugT[:, sl]),
                "saug": saug,
                "cbias": np.ascontiguousarray(
                    cbias[sl].reshape(N_TILES, 128).T
                ),
            }
        )
    return b, in_maps


def _run(x, supports, alphas, trace=False, **run_kwargs):
    b, in_maps = _prepare(x, supports, alphas)
    key = (b, bool(os.environ.get("BASS_ACT_ACCUM")))
    if key not in _compiled_cache:
        _compiled_cache[key] = _build(b)
    nc = _compiled_cache[key]
    res = run_bass_kernel_spmd(
        nc, in_maps, core_ids=list(range(N_CORES)), trace=trace, **run_kwargs
    )
    outs = [r["out"].T.reshape(-1) for r in res.results]
    return np.concatenate(outs).astype(np.float32), res


def kernel(x, supports, alphas):
    out, _ = _run(x, supports, alphas, trace=False)
    return out


# revision 21
# speedup vs baseline: 1.0107x; 1.0009x over previous
"""RBF-kernel SVM decision function on 8 TRN2 NeuronCores.

out[i] = sum_j alphas[j] * exp(-GAMMA * ||x[i] - supports[j]||^2)

Strategy (data-parallel over x rows, supports/alphas replicated):
  exponent e_ij = -g|x_i|^2 + (2g x_i . s_j) + (ln|a_j| - g|s_j|^2)
    - 2g x_i.s_j  : bf16 matmul, x-side scaled by 1/32 (exact pow2), s-side raw
    - j-term      : folded into the matmul as 2 extra contraction rows (hi/lo
                    bf16 split for ~fp24 accuracy), x-side rows = 1.0
    - i-term      : fp32 per-partition bias of the ACTIVATE(Exp)
  out_i = sum_{j: a_j>0} exp(e_ij) - sum_{j: a_j<0} exp(e_ij)
    - supports host-sorted so positive-alpha group comes first

Two reduction schemes:
  dve_accum (default, hybrid): per i-tile, one j-window is reduced by
    ACTIVATE(accum_out=...) in place on PSUM (one ACTIVATION_READ_ACCUMULATOR);
    the other three are written as fp16 to SBUF staging and reduced by the
    otherwise-idle DVE via tensor_scalar(accum_out=...) (1x-rate
    TENSOR_SCALAR_CACHE_REDUCE, ~1.75us/window, hidden under ScalarE). The
    sign-boundary window goes to the DVE side where an arbitrary split point
    just costs one extra op. ScalarE stays at its ~2us/window floor.
  act_accum (fallback, BASS_ACT_ACCUM=1): every window reduced by
    ACTIVATE(accum_out=...); costs an accumulator read per window plus a
    split ACTIVATE in the sign-boundary window (~7% slower overall).
"""

import os
import sys

for p in ("/opt/trn_rl_repo",):
    if p not in sys.path:
        sys.path.insert(0, p)

import numpy as np
import ml_dtypes

import concourse.bass as bass
import concourse.tile as tile
from concourse import bacc, mybir
from concourse.bass_utils import run_bass_kernel_spmd

N_CORES = 8
N = 16384
M = 8192
F = 64
GAMMA = 1.0 / F
N_LOC = N // N_CORES        # 2048 queries per core
N_TILES = N_LOC // 128      # 16 i-tiles of 128 queries
K_AUG = F + 2               # 66 contraction rows
W = 2048                    # j-window: 4 PSUM banks
NW = M // W                 # 4 windows per j sweep
MM_N = 512                  # matmul moving free dim (1 PSUM bank)
M_PAD = M + 256             # fp16 staging width (zero tail pad, mult of 4)

BF16 = mybir.dt.bfloat16
FP16 = mybir.dt.float16
F32 = mybir.dt.float32
bf16 = ml_dtypes.bfloat16

_compiled_cache = {}


def _build_common(nc, tc, cpool):
    """Input DRAM tensors, table-load warmer, and input DMAs (sync-ordered so
    the first window's operands land first)."""
    xaugT_d = nc.dram_tensor("xaugT", [K_AUG, N_LOC], BF16, kind="ExternalInput")
    saug_d = nc.dram_tensor("saug", [K_AUG, M], BF16, kind="ExternalInput")
    cbias_d = nc.dram_tensor("cbias", [128, N_TILES], F32, kind="ExternalInput")
    out_d = nc.dram_tensor("out", [128, N_TILES], F32, kind="ExternalOutput")

    # Dummy exp() on a zeroed tile: first in the ACT engine's program, so the
    # ~2.7us exp table load overlaps the input DMAs instead of stalling the
    # first real ACTIVATE.
    warm_act = cpool.tile([128, 1], F32)
    nc.gpsimd.memset(warm_act[:], 0.0)
    nc.scalar.activation(warm_act[:], warm_act[:], mybir.ActivationFunctionType.Exp)

    saug_sb = cpool.tile([K_AUG, M], BF16)
    nc.sync.dma_start(saug_sb[:, 0:W], saug_d.ap()[:, 0:W])
    xaugT_sb = cpool.tile([K_AUG, N_LOC], BF16)
    nc.sync.dma_start(xaugT_sb[:, 0:128], xaugT_d.ap()[:, 0:128])
    cbias_sb = cpool.tile([128, N_TILES], F32)
    nc.sync.dma_start(cbias_sb[:], cbias_d.ap()[:])
    for w in range(1, NW):
        nc.sync.dma_start(
            saug_sb[:, w * W : (w + 1) * W],
            saug_d.ap()[:, w * W : (w + 1) * W],
        )
    nc.sync.dma_start(xaugT_sb[:, 128:], xaugT_d.ap()[:, 128:])
    return xaugT_sb, saug_sb, cbias_sb, out_d


def _mm_windows(nc, t, ps_tile, w, xaugT_sb, saug_sb):
    for c in range(W // MM_N):
        nc.tensor.matmul(
            ps_tile[:, c * MM_N : (c + 1) * MM_N],
            xaugT_sb[:, t * 128 : (t + 1) * 128],
            saug_sb[:, w * W + c * MM_N : w * W + (c + 1) * MM_N],
            start=True,
            stop=True,
        )


def _build_dve_accum(b):
    """Hybrid reduction: one window per i-tile uses ACTIVATE(accum_out=...)
    (in-place on PSUM, one accumulator read); the other three are written as
    fp16 to SBUF staging and reduced by the otherwise-idle DVE with 1x
    TENSOR_SCALAR_CACHE_REDUCE ops. The sign-boundary window goes to the DVE,
    where an arbitrary split point costs only one extra small op."""
    nc = bacc.Bacc(
        "TRN2",
        target_bir_lowering=False,
        debug=False,
        enable_asserts=False,
        num_devices=N_CORES,
    )
    w_mix = b // W  # window containing the P/N boundary (b % W may be 0)
    act_w = 0 if w_mix != 0 else 1  # the one ACT-accum window, never mixed
    dve_ws = [w for w in range(NW) if w != act_w]

    def pieces_of(w):
        lo, hi = w * W, (w + 1) * W
        if b <= lo:
            return [(lo, hi, False)]
        if b >= hi:
            return [(lo, hi, True)]
        return [(lo, b, True), (b, hi, False)]

    n_pos = sum(1 for w in range(NW) for p in pieces_of(w) if p[2])
    n_neg = sum(1 for w in range(NW) for p in pieces_of(w) if not p[2])

    with tile.TileContext(nc) as tc:
        with (
            tc.tile_pool(name="const", bufs=1) as cpool,
            tc.tile_pool(name="acc", bufs=3) as apool,
            tc.tile_pool(name="stg", bufs=3) as spool,
            tc.tile_pool(name="psum", bufs=2, space="PSUM") as ppool,
        ):
            xaugT_sb, saug_sb, cbias_sb, out_d = _build_common(nc, tc, cpool)
            outT_sb = cpool.tile([128, N_TILES], F32)
            dvout = cpool.tile([128, M], FP16)

            for t in range(N_TILES):
                accP = apool.tile([128, max(n_pos, 1)], F32, tag="accP")
                accN = apool.tile([128, max(n_neg, 1)], F32, tag="accN")
                iP = iN = 0

                def acc_col(pos):
                    nonlocal iP, iN
                    if pos:
                        col = accP[:, iP : iP + 1]
                        iP += 1
                    else:
                        col = accN[:, iN : iN + 1]
                        iN += 1
                    return col

                # Last i-tile: ACT-accum everything (split at the sign
                # boundary) so no DVE reduce chain trails the final ACTIVATE.
                last = t == N_TILES - 1
                stg = spool.tile([128, M], FP16, tag="stg")
                for w in range(NW):
                    ps_tile = ppool.tile([128, W], F32, tag="E")
                    _mm_windows(nc, t, ps_tile, w, xaugT_sb, saug_sb)
                    if last or w == act_w:
                        for lo, hi, pos in pieces_of(w) if last else [
                            (w * W, (w + 1) * W, w * W < b)
                        ]:
                            nc.scalar.activation(
                                ps_tile[:, lo - w * W : hi - w * W],
                                ps_tile[:, lo - w * W : hi - w * W],
                                mybir.ActivationFunctionType.Exp,
                                bias=cbias_sb[:, t : t + 1],
                                accum_out=acc_col(pos),
                            )
                    else:
                        nc.scalar.activation(
                            stg[:, w * W : (w + 1) * W],
                            ps_tile[:],
                            mybir.ActivationFunctionType.Exp,
                            bias=cbias_sb[:, t : t + 1],
                        )
                if not last:
                    for w in dve_ws:
                        for lo, hi, pos in pieces_of(w):
                            nc.vector.tensor_scalar(
                                dvout[:, lo:hi],
                                stg[:, lo:hi],
                                1.0,
                                0.0,
                                mybir.AluOpType.mult,
                                mybir.AluOpType.add,
                                accum_out=acc_col(pos),
                            )
                sumP = apool.tile([128, 1], F32, tag="sumP")
                nc.vector.reduce_sum(sumP[:], accP[:, :iP], axis=mybir.AxisListType.X)
                sumN = apool.tile([128, 1], F32, tag="sumN")
                nc.vector.reduce_sum(sumN[:], accN[:, :iN], axis=mybir.AxisListType.X)
                nc.vector.tensor_sub(outT_sb[:, t : t + 1], sumP[:], sumN[:])

            nc.sync.dma_start(out_d.ap()[:], outT_sb[:])

    nc.compile()
    return nc


def _build_act_accum(b):
    nc = bacc.Bacc(
        "TRN2",
        target_bir_lowering=False,
        debug=False,
        enable_asserts=False,
        num_devices=N_CORES,
    )
    n_pos = sum(1 for w in range(NW) if w * W < b)
    n_neg = sum(1 for w in range(NW) if (w + 1) * W > b)

    with tile.TileContext(nc) as tc:
        with (
            tc.tile_pool(name="const", bufs=1) as cpool,
            tc.tile_pool(name="acc", bufs=3) as apool,
            tc.tile_pool(name="psum", bufs=2, space="PSUM") as ppool,
        ):
            xaugT_sb, saug_sb, cbias_sb, out_d = _build_common(nc, tc, cpool)
            outT_sb = cpool.tile([128, N_TILES], F32)

            for t in range(N_TILES):
                accP = apool.tile([128, max(n_pos, 1)], F32, tag="accP")
                accN = apool.tile([128, max(n_neg, 1)], F32, tag="accN")
                iP = iN = 0
                for w in range(NW):
                    ps_tile = ppool.tile([128, W], F32, tag="E")
                    _mm_windows(nc, t, ps_tile, w, xaugT_sb, saug_sb)
                    lo, hi = w * W, (w + 1) * W
                    if b <= lo:
                        pieces = [(lo, hi, False)]
                    elif b >= hi:
                        pieces = [(lo, hi, True)]
                    else:
                        pieces = [(lo, b, True), (b, hi, False)]
                    for plo, phi, pos in pieces:
                        if pos:
                            acc_col = accP[:, iP : iP + 1]
                            iP += 1
                        else:
                            acc_col = accN[:, iN : iN + 1]
                            iN += 1
                        nc.scalar.activation(
                            ps_tile[:, plo - lo : phi - lo],
                            ps_tile[:, plo - lo : phi - lo],
                            mybir.ActivationFunctionType.Exp,
                            bias=cbias_sb[:, t : t + 1],
                            accum_out=acc_col,
                        )
                sumP = apool.tile([128, 1], F32, tag="sumP")
                nc.vector.reduce_sum(sumP[:], accP[:, :iP], axis=mybir.AxisListType.X)
                sumN = apool.tile([128, 1], F32, tag="sumN")
                nc.vector.reduce_sum(sumN[:], accN[:, :iN], axis=mybir.AxisListType.X)
                nc.vector.tensor_sub(outT_sb[:, t : t + 1], sumP[:], sumN[:])

            nc.sync.dma_start(out_d.ap()[:], outT_sb[:])

    nc.compile()
    return nc


def _build(b):
    if os.environ.get("BASS_ACT_ACCUM"):
        return _build_act_accum(b)
    return _build_dve_accum(b)


def _prepare(x, supports, alphas):
    x = np.asarray(x, dtype=np.float32)
    supports = np.asarray(supports, dtype=np.float32)
    alphas = np.asarray(alphas, dtype=np.float32)

    a64 = alphas.astype(np.float64)
    s64 = supports.astype(np.float64)
    jterm = -GAMMA * (s64 * s64).sum(axis=1) + np.log(
        np.maximum(np.abs(a64), 1e-300)
    )

    pos = a64 > 0
    perm = np.concatenate([np.nonzero(pos)[0], np.nonzero(~pos)[0]])
    b = int(pos.sum())

    jt = jterm[perm]
    hi = jt.astype(bf16)
    lo = (jt - hi.astype(np.float64)).astype(bf16)

    saug = np.empty((K_AUG, M), dtype=bf16)
    saug[:F] = supports[perm].T.astype(bf16)
    saug[F] = hi
    saug[F + 1] = lo

    xaugT = np.ones((K_AUG, N), dtype=bf16)
    xaugT[:F] = (x.T / 32.0).astype(bf16)

    cbias = (-GAMMA * (x.astype(np.float64) ** 2).sum(axis=1)).astype(np.float32)

    in_maps = []
    for c in range(N_CORES):
        sl = slice(c * N_LOC, (c + 1) * N_LOC)
        in_maps.append(
            {
                "xaugT": np.ascontiguousarray(xaugT[:, sl]),
                "saug": saug,
                "cbias": np.ascontiguousarray(
                    cbias[sl].reshape(N_TILES, 128).T
                ),
            }
        )
    return b, in_maps


def _run(x, supports, alphas, trace=False, **run_kwargs):
    b, in_maps = _prepare(x, supports, alphas)
    key = (b, bool(os.environ.get("BASS_ACT_ACCUM")))
    if key not in _compiled_cache:
        _compiled_cache[key] = _build(b)
    nc = _compiled_cache[key]
    res = run_bass_kernel_spmd(
        nc, in_maps, core_ids=list(range(N_CORES)), trace=trace, **run_kwargs
    )
    outs = [r["out"].T.reshape(-1) for r in res.results]
    return np.concatenate(outs).astype(np.float32), res


def kernel(x, supports, alphas):
    out, _ = _run(x, supports, alphas, trace=False)
    return out


# revision 22
# speedup vs baseline: 1.0145x; 1.0038x over previous
"""RBF-kernel SVM decision function on 8 TRN2 NeuronCores.

out[i] = sum_j alphas[j] * exp(-GAMMA * ||x[i] - supports[j]||^2)

Strategy (data-parallel over x rows, supports/alphas replicated):
  exponent e_ij = -g|x_i|^2 + (2g x_i . s_j) + (ln|a_j| - g|s_j|^2)
    - 2g x_i.s_j  : bf16 matmul, x-side scaled by 1/32 (exact pow2), s-side raw
    - j-term      : folded into the matmul as 2 extra contraction rows (hi/lo
                    bf16 split for ~fp24 accuracy), x-side rows = 1.0
    - i-term      : fp32 per-partition bias of the ACTIVATE(Exp)
  out_i = sum_{j: a_j>0} exp(e_ij) - sum_{j: a_j<0} exp(e_ij)
    - supports host-sorted so positive-alpha group comes first

Two reduction schemes:
  dve_accum (default, hybrid): per i-tile, one j-window is reduced by
    ACTIVATE(accum_out=...) in place on PSUM (one ACTIVATION_READ_ACCUMULATOR);
    the other three are written as fp16 to SBUF staging and reduced by the
    otherwise-idle DVE via tensor_scalar(accum_out=...) (1x-rate
    TENSOR_SCALAR_CACHE_REDUCE, ~1.75us/window, hidden under ScalarE). The
    sign-boundary window goes to the DVE side where an arbitrary split point
    just costs one extra op. ScalarE stays at its ~2us/window floor.
  act_accum (fallback, BASS_ACT_ACCUM=1): every window reduced by
    ACTIVATE(accum_out=...); costs an accumulator read per window plus a
    split ACTIVATE in the sign-boundary window (~7% slower overall).
"""

import os
import sys

for p in ("/opt/trn_rl_repo",):
    if p not in sys.path:
        sys.path.insert(0, p)

import numpy as np
import ml_dtypes

import concourse.bass as bass
import concourse.tile as tile
from concourse import bacc, mybir
from concourse.bass_utils import run_bass_kernel_spmd

N_CORES = 8
N = 16384
M = 8192
F = 64
GAMMA = 1.0 / F
N_LOC = N // N_CORES        # 2048 queries per core
N_TILES = N_LOC // 128      # 16 i-tiles of 128 queries
K_AUG = F + 2               # 66 contraction rows
W = 2048                    # j-window: 4 PSUM banks
NW = M // W                 # 4 windows per j sweep
MM_N = 512                  # matmul moving free dim (1 PSUM bank)
M_PAD = M + 256             # fp16 staging width (zero tail pad, mult of 4)

BF16 = mybir.dt.bfloat16
FP16 = mybir.dt.float16
F32 = mybir.dt.float32
bf16 = ml_dtypes.bfloat16

_compiled_cache = {}


def _build_common(nc, tc, cpool):
    """Input DRAM tensors, table-load warmer, and input DMAs (sync-ordered so
    the first window's operands land first)."""
    xaugT_d = nc.dram_tensor("xaugT", [K_AUG, N_LOC], BF16, kind="ExternalInput")
    saug_d = nc.dram_tensor("saug", [K_AUG, M], BF16, kind="ExternalInput")
    cbias_d = nc.dram_tensor("cbias", [128, N_TILES], F32, kind="ExternalInput")
    out_d = nc.dram_tensor("out", [128, N_TILES], F32, kind="ExternalOutput")

    # Dummy exp() on a zeroed tile: first in the ACT engine's program, so the
    # ~2.7us exp table load overlaps the input DMAs instead of stalling the
    # first real ACTIVATE.
    warm_act = cpool.tile([128, 1], F32)
    nc.gpsimd.memset(warm_act[:], 0.0)
    nc.scalar.activation(warm_act[:], warm_act[:], mybir.ActivationFunctionType.Exp)

    saug_sb = cpool.tile([K_AUG, M], BF16)
    nc.sync.dma_start(saug_sb[:, 0:W], saug_d.ap()[:, 0:W])
    xaugT_sb = cpool.tile([K_AUG, N_LOC], BF16)
    nc.sync.dma_start(xaugT_sb[:, 0:128], xaugT_d.ap()[:, 0:128])
    cbias_sb = cpool.tile([128, N_TILES], F32)
    nc.sync.dma_start(cbias_sb[:], cbias_d.ap()[:])
    for w in range(1, NW):
        nc.sync.dma_start(
            saug_sb[:, w * W : (w + 1) * W],
            saug_d.ap()[:, w * W : (w + 1) * W],
        )
    nc.sync.dma_start(xaugT_sb[:, 128:], xaugT_d.ap()[:, 128:])
    return xaugT_sb, saug_sb, cbias_sb, out_d


def _mm_windows(nc, t, ps_tile, w, xaugT_sb, saug_sb):
    for c in range(W // MM_N):
        nc.tensor.matmul(
            ps_tile[:, c * MM_N : (c + 1) * MM_N],
            xaugT_sb[:, t * 128 : (t + 1) * 128],
            saug_sb[:, w * W + c * MM_N : w * W + (c + 1) * MM_N],
            start=True,
            stop=True,
        )


def _build_dve_accum(b):
    """Hybrid reduction: one window per i-tile uses ACTIVATE(accum_out=...)
    (in-place on PSUM, one accumulator read); the other three are written as
    fp16 to SBUF staging and reduced by the otherwise-idle DVE with 1x
    TENSOR_SCALAR_CACHE_REDUCE ops. The sign-boundary window goes to the DVE,
    where an arbitrary split point costs only one extra small op."""
    nc = bacc.Bacc(
        "TRN2",
        target_bir_lowering=False,
        debug=False,
        enable_asserts=False,
        num_devices=N_CORES,
    )
    w_mix = b // W  # window containing the P/N boundary (b % W may be 0)
    act_w = 0 if w_mix != 0 else 1  # the one ACT-accum window, never mixed
    dve_ws = [w for w in range(NW) if w != act_w]

    def pieces_of(w):
        lo, hi = w * W, (w + 1) * W
        if b <= lo:
            return [(lo, hi, False)]
        if b >= hi:
            return [(lo, hi, True)]
        return [(lo, b, True), (b, hi, False)]

    n_pos = sum(1 for w in range(NW) for p in pieces_of(w) if p[2])
    n_neg = sum(1 for w in range(NW) for p in pieces_of(w) if not p[2])

    with tile.TileContext(nc) as tc:
        with (
            tc.tile_pool(name="const", bufs=1) as cpool,
            tc.tile_pool(name="acc", bufs=3) as apool,
            tc.tile_pool(name="stg", bufs=3) as spool,
            tc.tile_pool(name="psum", bufs=2, space="PSUM") as ppool,
        ):
            xaugT_sb, saug_sb, cbias_sb, out_d = _build_common(nc, tc, cpool)
            outT_sb = cpool.tile([128, N_TILES], F32)
            dvout = cpool.tile([128, M], FP16)

            for t in range(N_TILES):
                accP = apool.tile([128, max(n_pos, 1)], F32, tag="accP")
                accN = apool.tile([128, max(n_neg, 1)], F32, tag="accN")
                iP = iN = 0

                def acc_col(pos):
                    nonlocal iP, iN
                    if pos:
                        col = accP[:, iP : iP + 1]
                        iP += 1
                    else:
                        col = accN[:, iN : iN + 1]
                        iN += 1
                    return col

                # Last i-tile: ACT-accum everything (split at the sign
                # boundary) so no DVE reduce chain trails the final ACTIVATE.
                # Even i-tiles: all four windows reduced on the DVE (no
                # accumulator read on ScalarE); odd i-tiles keep one ACT-accum
                # window so the DVE stays below the ScalarE pace.
                last = t == N_TILES - 1
                if last:
                    act_set = set(range(NW))
                elif t % 2 == 0:
                    act_set = set()
                else:
                    act_set = {act_w}
                stg = spool.tile([128, M], FP16, tag="stg")
                for w in range(NW):
                    ps_tile = ppool.tile([128, W], F32, tag="E")
                    _mm_windows(nc, t, ps_tile, w, xaugT_sb, saug_sb)
                    if w in act_set:
                        for lo, hi, pos in pieces_of(w):
                            nc.scalar.activation(
                                ps_tile[:, lo - w * W : hi - w * W],
                                ps_tile[:, lo - w * W : hi - w * W],
                                mybir.ActivationFunctionType.Exp,
                                bias=cbias_sb[:, t : t + 1],
                                accum_out=acc_col(pos),
                            )
                    else:
                        nc.scalar.activation(
                            stg[:, w * W : (w + 1) * W],
                            ps_tile[:],
                            mybir.ActivationFunctionType.Exp,
                            bias=cbias_sb[:, t : t + 1],
                        )
                for w in range(NW):
                    if w in act_set:
                        continue
                    for lo, hi, pos in pieces_of(w):
                        nc.vector.tensor_scalar(
                            dvout[:, lo:hi],
                            stg[:, lo:hi],
                            1.0,
                            0.0,
                            mybir.AluOpType.mult,
                            mybir.AluOpType.add,
                            accum_out=acc_col(pos),
                        )
                sumP = apool.tile([128, 1], F32, tag="sumP")
                nc.vector.reduce_sum(sumP[:], accP[:, :iP], axis=mybir.AxisListType.X)
                sumN = apool.tile([128, 1], F32, tag="sumN")
                nc.vector.reduce_sum(sumN[:], accN[:, :iN], axis=mybir.AxisListType.X)
                nc.vector.tensor_sub(outT_sb[:, t : t + 1], sumP[:], sumN[:])

            nc.sync.dma_start(out_d.ap()[:], outT_sb[:])

    nc.compile()
    return nc


def _build_act_accum(b):
    nc = bacc.Bacc(
        "TRN2",
        target_bir_lowering=False,
        debug=False,
        enable_asserts=False,
        num_devices=N_CORES,
    )
    n_pos = sum(1 for w in range(NW) if w * W < b)
    n_neg = sum(1 for w in range(NW) if (w + 1) * W > b)

    with tile.TileContext(nc) as tc:
        with (
            tc.tile_pool(name="const", bufs=1) as cpool,
            tc.tile_pool(name="acc", bufs=3) as apool,
            tc.tile_pool(name="psum", bufs=2, space="PSUM") as ppool,
        ):
            xaugT_sb, saug_sb, cbias_sb, out_d = _build_common(nc, tc, cpool)
            outT_sb = cpool.tile([128, N_TILES], F32)

            for t in range(N_TILES):
                accP = apool.tile([128, max(n_pos, 1)], F32, tag="accP")
                accN = apool.tile([128, max(n_neg, 1)], F32, tag="accN")
                iP = iN = 0
                for w in range(NW):
                    ps_tile = ppool.tile([128, W], F32, tag="E")
                    _mm_windows(nc, t, ps_tile, w, xaugT_sb, saug_sb)
                    lo, hi = w * W, (w + 1) * W
                    if b <= lo:
                        pieces = [(lo, hi, False)]
                    elif b >= hi:
                        pieces = [(lo, hi, True)]
                    else:
                        pieces = [(lo, b, True), (b, hi, False)]
                    for plo, phi, pos in pieces:
                        if pos:
                            acc_col = accP[:, iP : iP + 1]
                            iP += 1
                        else:
                            acc_col = accN[:, iN : iN + 1]
                            iN += 1
                        nc.scalar.activation(
                            ps_tile[:, plo - lo : phi - lo],
                            ps_tile[:, plo - lo : phi - lo],
                            mybir.ActivationFunctionType.Exp,
                            bias=cbias_sb[:, t : t + 1],
                            accum_out=acc_col,
                        )
                sumP = apool.tile([128, 1], F32, tag="sumP")
                nc.vector.reduce_sum(sumP[:], accP[:, :iP], axis=mybir.AxisListType.X)
                sumN = apool.tile([128, 1], F32, tag="sumN")
                nc.vector.reduce_sum(sumN[:], accN[:, :iN], axis=mybir.AxisListType.X)
                nc.vector.tensor_sub(outT_sb[:, t : t + 1], sumP[:], sumN[:])

            nc.sync.dma_start(out_d.ap()[:], outT_sb[:])

    nc.compile()
    return nc


def _build(b):
    if os.environ.get("BASS_ACT_ACCUM"):
        return _build_act_accum(b)
    return _build_dve_accum(b)


def _prepare(x, supports, alphas):
    x = np.asarray(x, dtype=np.float32)
    supports = np.asarray(supports, dtype=np.float32)
    alphas = np.asarray(alphas, dtype=np.float32)

    a64 = alphas.astype(np.float64)
    s64 = supports.astype(np.float64)
    jterm = -GAMMA * (s64 * s64).sum(axis=1) + np.log(
        np.maximum(np.abs(a64), 1e-300)
    )

    pos = a64 > 0
    perm = np.concatenate([np.nonzero(pos)[0], np.nonzero(~pos)[0]])
    b = int(pos.sum())

    jt = jterm[perm]
    hi = jt.astype(bf16)
    lo = (jt - hi.astype(np.float64)).astype(bf16)

    saug = np.empty((K_AUG, M), dtype=bf16)
    saug[:F] = supports[perm].T.astype(bf16)
    saug[F] = hi
    saug[F + 1] = lo

    xaugT = np.ones((K_AUG, N), dtype=bf16)
    xaugT[:F] = (x.T / 32.0).astype(bf16)

    cbias = (-GAMMA * (x.astype(np.float64) ** 2).sum(axis=1)).astype(np.float32)

    in_maps = []
    for c in range(N_CORES):
        sl = slice(c * N_LOC, (c + 1) * N_LOC)
        in_maps.append(
            {
                "xaugT": np.ascontiguousarray(xaugT[:, sl]),
                "saug": saug,
                "cbias": np.ascontiguousarray(
                    cbias[sl].reshape(N_TILES, 128).T
                ),
            }
        )
    return b, in_maps


def _run(x, supports, alphas, trace=False, **run_kwargs):
    b, in_maps = _prepare(x, supports, alphas)
    key = (b, bool(os.environ.get("BASS_ACT_ACCUM")))
    if key not in _compiled_cache:
        _compiled_cache[key] = _build(b)
    nc = _compiled_cache[key]
    res = run_bass_kernel_spmd(
        nc, in_maps, core_ids=list(range(N_CORES)), trace=trace, **run_kwargs
    )
    outs = [r["out"].T.reshape(-1) for r in res.results]
    return np.concatenate(outs).astype(np.float32), res


def kernel(x, supports, alphas):
    out, _ = _run(x, supports, alphas, trace=False)
    return out
